# revision 1
# baseline (speedup 1.0000x reference)
"""Causal multi-head attention (B=4, H=16, S=2048, D=64) on 8 Trainium2 cores.

Sharding: B*H = 64 independent attention problems, 8 heads per core.

Per-core design (v3 — single-fp16 matmuls, scalar-engine-saturating pipeline):
- Heads in pairs (A at PE rows 0:64, B at 64:128); the two heads' QK matmuls
  run concurrently on disjoint PE row groups.
- fp16 throughout the matmuls (rel tolerance is 2e-2; fp16 lands ~1e-3):
  one QK matmul and one PV matmul per (k-chunk, head) work item.
- S^T computed directly in [k, q] layout (lhsT = K^T chunk, rhs = Q^T block)
  so no P transposes are needed; Q^T/K^T produced by DMA xbar transposes.
- q-blocks of 256 columns; per block b only k-chunks 0..2b+1 are computed
  (exact causal coverage at 128x256 granularity). The last k-chunk (j2=1)
  restricts its matmul to the valid upper 128 columns; the 128-wide diagonal
  triangle is zeroed with one shared fp16 mask on the vector engine.
- Work items (b, cc, hh) stream in groups of 6; each group's scores live in a
  3-bank PSUM tile [128, 1536] (double-buffered -> the exp ACT on the scalar
  engine, the bottleneck at ~150us/core, runs back-to-back) and one
  activation(Exp, scale=1/8) per group converts to fp16 P.
- PV accumulates [65, 512] in a single PSUM bank per q-block (head A cols
  0:256, head B 256:512; ones column of V gives softmax denominators free).
  Single start=True on the block's first PV: start marks the whole bank
  pending-zero, each matmul overwrites-where-pending / accumulates-otherwise.
- Finalize per block: 4 PE transposes [65,128] -> [128, 4*65], reciprocal of
  the den rows, per-partition scalar multiply, contiguous DMA out.
"""
import numpy as np

B, H, S, D = 4, 16, 2048, 64
NCORES = 8
HPC = B * H // NCORES      # 8 heads per core
P = 128
QBLK = 256
NT = S // P                # 16 k-chunks per head
NBLK = S // QBLK           # 8 q blocks
NPAIR = HPC // 2           # 4 head pairs per core
GROUP = 6                  # work items per ACT group (3 PSUM banks)

_cache = {}


def _build(reps=1, loop=0, acc_packed=True, st_bufs=2, do_pv=True,
           do_fin=True, do_qk=True, do_act=True, do_mask=True,
           ones_memset=True, qk_mode="normal", split_prologue=False,
           batched_out=True, mask_gpsimd=False, pt_bufs=4,
           out_engine="gpsimd"):
    from contextlib import ExitStack
    import concourse.bacc as bacc
    import concourse.tile as tile
    import concourse.mybir as mybir
    from concourse.masks import make_identity

    f32 = mybir.dt.float32
    f16 = mybir.dt.float16
    AF = mybir.ActivationFunctionType
    ET = mybir.EngineType

    nc = bacc.Bacc("TRN2", target_bir_lowering=False, debug=False,
                   num_devices=NCORES)
    Qd = nc.dram_tensor("Q", (HPC, S, D), f32, kind="ExternalInput")
    Kd = nc.dram_tensor("K", (HPC, S, D), f32, kind="ExternalInput")
    Vd = nc.dram_tensor("V", (HPC, S, D), f32, kind="ExternalInput")
    Od = nc.dram_tensor("O", (HPC, S, D), f32, kind="ExternalOutput")

    with tile.TileContext(nc) as tc, ExitStack() as ctx:
        consts = ctx.enter_context(tc.tile_pool(name="consts", bufs=1))
        raw = ctx.enter_context(tc.tile_pool(name="raw", bufs=2))
        cvt = ctx.enter_context(tc.tile_pool(name="cvt", bufs=2))
        qk = ctx.enter_context(tc.tile_pool(name="qk", bufs=2))
        ptp = ctx.enter_context(tc.tile_pool(name="ptp", bufs=pt_bufs))
        fin = ctx.enter_context(tc.tile_pool(name="fin", bufs=3))
        st_ps = ctx.enter_context(tc.tile_pool(name="st_ps", bufs=st_bufs, space="PSUM"))
        acc_ps = ctx.enter_context(tc.tile_pool(name="acc_ps", bufs=1, space="PSUM"))
        tr_ps = ctx.enter_context(tc.tile_pool(name="tr_ps", bufs=1, space="PSUM"))

        ident = consts.tile([128, 128], f32)
        make_identity(nc, ident[:])
        # tri[r, c] = 1.0 iff c >= r (keep); used on the 128-wide diagonal
        # triangle of every diagonal k-chunk.
        trif = consts.tile([128, 128], f32)
        nc.gpsimd.memset(trif[:], 1.0)
        nc.gpsimd.affine_select(
            out=trif[:], in_=trif[:], compare_op=mybir.AluOpType.is_ge,
            fill=0.0, base=0, pattern=[[1, 128]], channel_multiplier=-1)
        tri = consts.tile([128, 128], f16)
        nc.vector.tensor_copy(tri[:], trif[:])
        onesf = consts.tile([128, 2 * NT], f32)
        nc.vector.memset(onesf[:], 1.0)

        # Work items for one head pair: (block, k-chunk, head, col offset).
        # Head-major within a block: the two matmuls sharing a PSUM bank
        # (stream positions 2k, 2k+1) must use the SAME PE row group — two
        # concurrent row-group-disjoint matmuls draining into one bank are
        # fatal on HW. Per-head chunk counts are even, so bank pairs never
        # mix heads in this order.
        items = []
        for b in range(NBLK):
            for hh in range(2):
                for cc in range(2 * b + 2):
                    off = 128 if cc == 2 * b + 1 else 0
                    items.append((b, cc, hh, off))
        assert len(items) % GROUP == 0
        groups = [items[i:i + GROUP] for i in range(0, len(items), GROUP)]
        # group index after which block b's accumulation is complete
        last_group_of_block = {}
        for gi, grp in enumerate(groups):
            for (b, cc, hh, off) in grp:
                if cc == 2 * b + 1 and hh == 1:
                    last_group_of_block[b] = gi

        def emit_qk(grp, kt, qt, st):
            # Full 256-col width always: for the last diagonal chunk (off=128)
            # cols [0:128) hold garbage scores (k > q), bounded by exp(|s|/8);
            # the PV matmul restricts N so they are never read.
            if not do_qk:
                return
            for i, (b, cc, hh, off) in enumerate(grp):
                nc.tensor.matmul(
                    st[:, i * 256:(i + 1) * 256],
                    kt[hh * 64:(hh + 1) * 64, cc * 128:(cc + 1) * 128],
                    qt[hh * 64:(hh + 1) * 64, b * QBLK:(b + 1) * QBLK],
                    start=True, stop=True)

        def emit_act_mask_pv(grp, st, vh_v, accs):
            if not (do_qk and do_act):
                return
            pt = ptp.tile([128, GROUP * 256], f16, tag="pt", name="pt")
            nc.scalar.activation(pt[:], st[:], AF.Exp, scale=0.125)
            for i, (b, cc, hh, off) in enumerate(grp):
                j2 = cc - 2 * b
                if j2 >= 0 and do_mask:  # diagonal chunk: zero the triangle
                    a = i * 256 + 128 * j2
                    eng = nc.gpsimd if mask_gpsimd else nc.vector
                    eng.tensor_mul(pt[:, a:a + 128], pt[:, a:a + 128],
                                   tri[:])
            if not do_pv:
                return
            for i, (b, cc, hh, off) in enumerate(grp):
                if acc_packed:
                    nc.tensor.matmul(
                        accs[b][:, hh * 256 + off:(hh + 1) * 256],
                        vh_v[:, hh, cc, :],
                        pt[:, i * 256 + off:(i + 1) * 256],
                        start=(cc == 0 and hh == 0),
                        stop=(cc == 2 * b + 1 and hh == 1))
                else:
                    nc.tensor.matmul(
                        accs[b][hh][:, off:256],
                        vh_v[:, hh, cc, :],
                        pt[:, i * 256 + off:(i + 1) * 256],
                        start=(cc == 0), stop=(cc == 2 * b + 1))

        def emit_finalize(hA, b, accs, o_stage):
            if not do_fin:
                return
            osb = fin.tile([65, 512], f32, tag="osb", name="osb")
            if acc_packed:
                nc.vector.tensor_copy(osb[:], accs[b][:])
            else:
                for hh in range(2):
                    nc.vector.tensor_copy(osb[:, hh * 256:(hh + 1) * 256],
                                          accs[b][hh][:])
            ot = tr_ps.tile([128, 260], f32, tag="tr", name="ot")
            for k in range(4):  # k = 2*hh + j (j = q sub-tile of 128)
                nc.tensor.transpose(
                    ot[:, k * 65:(k + 1) * 65],
                    osb[:, k * 128:(k + 1) * 128],
                    ident[0:65, 0:65])
            recip = fin.tile([128, 4], f32, tag="recip", name="recip")
            nc.vector.reciprocal(
                recip[:],
                ot[:].rearrange("p (k e) -> p k e", e=65)[:, :, 64])
            for k in range(4):  # o_stage cols are (hh, b, j, d) head-major
                hh, j = divmod(k, 2)
                a = hh * (NBLK * 128) + b * 128 + j * 64
                nc.vector.tensor_scalar_mul(
                    o_stage[:, a:a + 64],
                    ot[:, k * 65:k * 65 + 64],
                    recip[:, k:k + 1])
            if not batched_out:
                for hh in range(2):
                    a = hh * (NBLK * 128) + b * 128
                    nc.sync.dma_start(
                        Od[hA + hh, b * QBLK:(b + 1) * QBLK, :]
                        .rearrange("(j p) d -> p j d", p=P),
                        o_stage[:, a:a + 128]
                        .rearrange("p (j d) -> p j d", j=2))

        def prologue(pair):
            hA = 2 * pair
            # ---- loads + fp16 converts + batched xbar transposes ----
            # The batched transpose does all 16 stacked [128,128] tile
            # transposes in one instruction: out[p, n, c] = in[c, n*128 + p].
            qraw = raw.tile([128, NT * 2 * 64], f32)
            kraw = raw.tile([128, NT * 2 * 64], f32)
            vf = raw.tile([128, 2 * NT * 64], f32)
            q16 = cvt.tile([128, NT * 128], f16)
            k16 = cvt.tile([128, NT * 128], f16)
            qt = qk.tile([128, S], f16)
            kt = qk.tile([128, S], f16)
            nhalves = 2 if split_prologue else 1
            HNT = NT // nhalves
            for half in range(nhalves):
                tsl = slice(half * HNT, (half + 1) * HNT)
                csl = slice(half * HNT * 128, (half + 1) * HNT * 128)
                for src_d, rawt, c16, ct in ((Qd, qraw, q16, qt),
                                             (Kd, kraw, k16, kt)):
                    raw_v = rawt[:].rearrange("p (n h d) -> p n h d",
                                              n=NT, h=2)
                    for hh in range(2):
                        nc.sync.dma_start(
                            raw_v[:, tsl, hh, :],
                            src_d[hA + hh,
                                  half * HNT * P:(half + 1) * HNT * P, :]
                            .rearrange("(n p) d -> p n d", p=P))
                    nc.vector.tensor_copy(c16[:, csl], rawt[:, csl])
                    nc.sync.dma_start_transpose(
                        ct[:, csl].rearrange("p (n c) -> p n c", n=HNT),
                        c16[:, csl])
            # V is loaded after Q/K (first used only after the first exp).
            for hh in range(2):
                nc.sync.dma_start(
                    vf[:].rearrange("p (h n d) -> p h n d", h=2, n=NT)[:, hh, :, :],
                    Vd[hA + hh, :, :].rearrange("(n p) d -> p n d", p=P))
            vh = cvt.tile([128, 2 * NT * 65], f16)
            vh_v = vh[:].rearrange("p (h n e) -> p h n e", h=2, n=NT)
            nc.vector.tensor_copy(
                vh_v[:, :, :, 0:64],
                vf[:].rearrange("p (h n d) -> p h n d", h=2, n=NT))
            if ones_memset:
                nc.vector.memset(vh_v[:, :, :, 64:65], 1.0)
            else:
                nc.vector.tensor_copy(
                    vh_v[:, :, :, 64:65],
                    onesf[:].rearrange("p (h n) -> p h n", h=2)[:, :, :, None])
            return qt, kt, vh_v

        def body():
            # Prologues are emitted one pair ahead so the next pair's
            # loads/converts/transposes land ahead of the current pair's
            # masks/finalize work in the SP and DVE FIFOs (engine queues are
            # strict FIFO; emission order is execution order per engine).
            staged = {0: prologue(0)}
            if NPAIR > 1:
                staged[1] = prologue(1)
            for pair in range(NPAIR):
                hA = 2 * pair
                qt, kt, vh_v = staged.pop(pair)

                # ---- attention stream: 1-group software pipeline lag ----
                o_stage = fin.tile([128, NBLK * 256], f32, tag="ostage",
                                   name="ostage")
                accs = {}
                sts = {}
                prev = None
                for gi, grp in enumerate(groups):
                    for (b, cc, hh, off) in grp:
                        if b not in accs:
                            if acc_packed:
                                accs[b] = acc_ps.tile([65, 512], f32,
                                                      tag="acc", name="acc")
                            else:
                                accs[b] = [
                                    acc_ps.tile([65, 256], f32,
                                                tag=f"acc{h2}", name="acc")
                                    for h2 in range(2)]
                    st = st_ps.tile([128, GROUP * 256], f32, tag="st", name="st")
                    sts[gi] = st
                    emit_qk(grp, kt, qt, st)
                    if prev is not None:
                        emit_act_mask_pv(groups[prev], sts.pop(prev), vh_v, accs)
                        if prev in done_blocks:
                            emit_finalize(hA, done_blocks[prev], accs, o_stage)
                    prev = gi
                emit_act_mask_pv(groups[prev], sts.pop(prev), vh_v, accs)
                emit_finalize(hA, NBLK - 1, accs, o_stage)
                # ---- batched output DMA: one per head, on the (idle)
                # gpsimd SWDGE queue. On the SP HWDGE FIFO it would block
                # the next pair's loads/transposes until this pair's
                # finalizes complete, serializing the pair pipeline.
                if do_fin and batched_out:
                    for hh in range(2):
                        getattr(nc, out_engine).dma_start(
                            Od[hA + hh, :, :]
                            .rearrange("(b j p) d -> p b j d", p=P, j=2),
                            o_stage[:, hh * NBLK * 128:(hh + 1) * NBLK * 128]
                            .rearrange("p (b j d) -> p b j d", b=NBLK, j=2))
                if pair + 2 < NPAIR:
                    staged[pair + 2] = prologue(pair + 2)

        done_blocks = {gi: b for b, gi in last_group_of_block.items()
                       if b != NBLK - 1}

        if loop:
            with tc.For_i(0, loop, 1,
                          hint_engines=(ET.PE, ET.Activation, ET.DVE, ET.SP,
                                        ET.Pool),
                          staggered_reset=True):
                for _ in range(reps):
                    body()
        else:
            for _ in range(reps):
                body()

    nc.compile()
    return nc


def _get_nc():
    if "nc" not in _cache:
        _cache["nc"] = _build()
    return _cache["nc"]


def kernel(Q, K, V):
    from concourse.bass_utils import run_bass_kernel_spmd

    Q = np.ascontiguousarray(np.asarray(Q, dtype=np.float32)).reshape(B * H, S, D)
    K = np.ascontiguousarray(np.asarray(K, dtype=np.float32)).reshape(B * H, S, D)
    V = np.ascontiguousarray(np.asarray(V, dtype=np.float32)).reshape(B * H, S, D)

    nc = _get_nc()
    in_maps = [
        {"Q": Q[c * HPC:(c + 1) * HPC],
         "K": K[c * HPC:(c + 1) * HPC],
         "V": V[c * HPC:(c + 1) * HPC]}
        for c in range(NCORES)
    ]
    res = run_bass_kernel_spmd(nc, in_maps, core_ids=list(range(NCORES)))
    out = np.concatenate([res.results[c]["O"] for c in range(NCORES)], axis=0)
    return out.reshape(B, H, S, D)

